# revision 10
# baseline (speedup 1.0000x reference)
"""FP8 fake-quant matmul on 8 TRN2 NeuronCores.

Computes reference semantics:
    w_dq = fq(weight, s_w);  x_dq = fq(x.reshape(-1,K), s_x)
    out  = (x_dq @ w_dq).reshape(B, S, N)
where fq(t, s) = clip(t*s, +-448) round-tripped through float8_e4m3fn (OCP),
s = 448 / amax(|t|).

Device strategy (data-parallel over rows M = B*S, 8 shards):
  Launch A: per-core partial amax of x shard and weight shard -> host combines
            to the exact global fp32 amax / scales.
  Launch B: per-core quantize (TRN e4m3 at HALF scale: TRN fp8e4 max-normal is
            240, not 448 -- x*s/2 <= 224 needs no clip and rounds identically
            to OCP at full scale), PE-transpose x tiles, fp8 matmul accumulate
            fp32 in PSUM, dequant-scale, write out.
"""

import sys

for _p in ("/opt/trn_rl_repo", "/root/.axon_site"):
    if _p not in sys.path:
        sys.path.insert(0, _p)

import numpy as np

import concourse.bass as bass  # noqa: F401  (registers engine classes)
import concourse.tile as tile
from concourse import bacc, mybir
from concourse.bass_utils import run_bass_kernel_spmd
from concourse.masks import make_identity

# Problem shapes (hardcoded per spec)
B, S, K, N = 8, 2048, 4096, 4096
NCORES = 8
MS = (B * S) // NCORES  # 2048 rows of x per core
WS = K // NCORES  # 512 rows of weight per core (amax sharding)
P = 128
FP32 = mybir.dt.float32
FP8 = mybir.dt.float8e4
FP8_MAX = np.float32(448.0)

_CACHE = {}


def _build_amax():
    nc = bacc.Bacc(None, target_bir_lowering=False, debug=False)
    xs = nc.declare_dram_parameter("xs", [MS, K], FP32, isOutput=False)
    ws = nc.declare_dram_parameter("ws", [WS, K], FP32, isOutput=False)
    pm = nc.declare_dram_parameter("pm", [P, 2], FP32, isOutput=True)
    nxt = MS // P  # 16
    nwt = WS // P  # 4
    with tile.TileContext(nc) as tc:
        with (
            tc.tile_pool(name="io", bufs=4) as io,
            tc.tile_pool(name="st", bufs=1) as stp,
        ):
            st = stp.tile([P, nxt + nwt], FP32)
            fin = stp.tile([P, 2], FP32)
            xt = xs[:].rearrange("(t p) k -> t p k", p=P)
            wt = ws[:].rearrange("(t p) k -> t p k", p=P)
            for i in range(nxt):
                t = io.tile([P, K], FP32, tag="io")
                nc.sync.dma_start(out=t[:], in_=xt[i])
                nc.vector.reduce_max(
                    st[:, i : i + 1], t[:], axis=mybir.AxisListType.X,
                    apply_absolute_value=True,
                )
            for i in range(nwt):
                t = io.tile([P, K], FP32, tag="io")
                nc.sync.dma_start(out=t[:], in_=wt[i])
                nc.vector.reduce_max(
                    st[:, nxt + i : nxt + i + 1], t[:], axis=mybir.AxisListType.X,
                    apply_absolute_value=True,
                )
            nc.vector.reduce_max(
                fin[:, 0:1], st[:, 0:nxt], axis=mybir.AxisListType.X
            )
            nc.vector.reduce_max(
                fin[:, 1:2], st[:, nxt : nxt + nwt], axis=mybir.AxisListType.X
            )
            nc.sync.dma_start(out=pm[:], in_=fin[:])
    nc.compile()
    return nc


def _build_main():
    """Launch B: quantize + DoubleRow fp8 matmul.

    Takes x pre-transposed on the host (xt = x_shard.T, [K, MS] row-major) so
    both operands DMA with k on the partition axis; no on-chip transposes.
    """
    nc = bacc.Bacc(None, target_bir_lowering=False, debug=False)
    xT = nc.declare_dram_parameter("xT", [K, MS], FP32, isOutput=False)
    w = nc.declare_dram_parameter("w", [K, N], FP32, isOutput=False)
    sc = nc.declare_dram_parameter("sc", [1, 8], FP32, isOutput=False)
    out = nc.declare_dram_parameter("out", [MS, N], FP32, isOutput=True)
    MT, KT = MS // P, K // P  # 16, 32
    CT = KT // 2  # 16 DoubleRow chunks of 256 contraction rows
    NB = 512  # psum bank width (fp32)
    NT = N // NB  # 8 column sweeps
    MB = 512  # x m-strip width
    MST = MS // MB  # 4 strips
    DR = mybir.MatmulPerfMode.DoubleRow
    with tile.TileContext(nc) as tc:
        with (
            tc.tile_pool(name="const", bufs=1) as cst,
            tc.tile_pool(name="wf", bufs=8) as wfp,
            tc.tile_pool(name="wq", bufs=3 * CT) as wqp,
            tc.tile_pool(name="xf", bufs=8) as xfp,
            tc.tile_pool(name="xq", bufs=CT) as xqp,
            tc.tile_pool(name="ob", bufs=4) as obp,
            tc.tile_pool(name="mps", bufs=6, space="PSUM") as mpsp,
        ):
            scs = cst.tile([P, 8], FP32)
            nc.sync.dma_start(out=scs[:], in_=sc[:].to_broadcast([P, 8]))
            sxs = scs[:, 0:1]  # s_x / 2
            sws = scs[:, 1:2]  # s_w / 2
            dqs = scs[:, 2:3]  # 4 / (s_x * s_w) with reference rounding

            # DoubleRow pairing: chunk c, plane i, partition p <-> k row
            # c*256 + i*128 + p, for both operands.
            w4 = w[:].rearrange("(c i p) n -> c p i n", i=2, p=P)  # [16,128,2,N]
            x4 = xT[:].rearrange("(c i p) m -> c p i m", i=2, p=P)  # [16,128,2,MS]
            ot = out[:].rearrange("(t p) n -> t p n", p=P)

            # Quantized x^T: resident, one tile per 256-row chunk.
            xqs = [
                xqp.tile([P, 2, MS], FP8, tag="xq", name=f"xq_{c}")
                for c in range(CT)
            ]

            def emit_xstrip(ms):
                # loads + quantizes x^T columns [ms*MB, (ms+1)*MB) for all chunks
                for c in range(CT):
                    xf = xfp.tile([P, 2, MB], FP32, tag="xf", name=f"xf_{ms}_{c}")
                    nc.sync.dma_start(
                        out=xf[:], in_=x4[c][:, :, ms * MB : (ms + 1) * MB]
                    )
                    if c % 2:
                        nc.scalar.mul(
                            xqs[c][:, :, ms * MB : (ms + 1) * MB], xf[:], sxs
                        )
                    else:
                        nc.vector.tensor_scalar_mul(
                            xqs[c][:, :, ms * MB : (ms + 1) * MB], xf[:], sxs
                        )

            wgroups = {}

            def emit_wgroup(j):
                tiles = []
                for c in range(CT):
                    wf = wfp.tile([P, 2, NB], FP32, tag="wf", name=f"wf_{j}_{c}")
                    nc.sync.dma_start(
                        out=wf[:], in_=w4[c][:, :, j * NB : (j + 1) * NB]
                    )
                    wq = wqp.tile([P, 2, NB], FP8, tag="wq", name=f"wq_{j}_{c}")
                    nc.scalar.mul(wq[:, :, :], wf[:, :, :], sws)
                    tiles.append(wq)
                wgroups[j] = tiles

            emit_xstrip(0)
            emit_wgroup(0)
            emit_xstrip(1)
            emit_xstrip(2)
            emit_xstrip(3)
            emit_wgroup(1)
            for j in range(NT):
                for m in range(MT):
                    psum = mpsp.tile([P, NB], FP32, tag="mps", name=f"mps_{j}_{m}")
                    for c in range(CT):
                        nc.tensor.matmul(
                            psum[:],
                            xqs[c][:, :, m * P : (m + 1) * P],
                            wgroups[j][c][:, :, :],
                            start=(c == 0),
                            stop=(c == CT - 1),
                            perf_mode=DR,
                        )
                    if m == 8 and j + 2 < NT:
                        emit_wgroup(j + 2)
                    ob = obp.tile([P, NB], FP32, tag="ob", name=f"ob_{j}_{m}")
                    nc.vector.tensor_scalar_mul(ob[:], psum[:], dqs)
                    nc.sync.dma_start(out=ot[m, :, j * NB : (j + 1) * NB], in_=ob[:])
                del wgroups[j]
    nc.compile()
    return nc


def _get(name, builder):
    if name not in _CACHE:
        _CACHE[name] = builder()
    return _CACHE[name]


def kernel(x: np.ndarray, weight: np.ndarray) -> np.ndarray:
    x = np.ascontiguousarray(np.asarray(x, dtype=np.float32))
    weight = np.ascontiguousarray(np.asarray(weight, dtype=np.float32))
    assert x.shape == (B, S, K) and weight.shape == (K, N)
    x2d = x.reshape(B * S, K)

    core_ids = list(range(NCORES))
    x_shards = [x2d[c * MS : (c + 1) * MS] for c in core_ids]
    w_shards = [weight[c * WS : (c + 1) * WS] for c in core_ids]

    # ---- Launch A: partial amax ----
    nc_a = _get("amax", _build_amax)
    res_a = run_bass_kernel_spmd(
        nc_a,
        [{"xs": x_shards[c], "ws": w_shards[c]} for c in core_ids],
        core_ids,
    )
    pms = np.stack([res_a.results[c]["pm"] for c in core_ids])  # [8, 128, 2]
    amax_x = np.float32(pms[:, :, 0].max())
    amax_w = np.float32(pms[:, :, 1].max())

    # Exact reference scale arithmetic (fp32 throughout)
    s_x = FP8_MAX / np.maximum(amax_x, np.float32(1e-12))
    s_w = FP8_MAX / np.maximum(amax_w, np.float32(1e-12))
    r_x = np.float32(1.0) / s_x
    r_w = np.float32(1.0) / s_w
    dq = np.float32(4.0) * r_x * r_w
    scales = np.zeros((1, 8), np.float32)
    scales[0, 0] = s_x * np.float32(0.5)
    scales[0, 1] = s_w * np.float32(0.5)
    scales[0, 2] = dq

    # ---- Launch B: quantize + matmul (x pre-transposed per shard on host) ----
    xT_shards = [np.ascontiguousarray(s.T) for s in x_shards]
    nc_b = _get("main", _build_main)
    res_b = run_bass_kernel_spmd(
        nc_b,
        [{"xT": xT_shards[c], "w": weight, "sc": scales} for c in core_ids],
        core_ids,
    )
    out = np.concatenate([res_b.results[c]["out"] for c in core_ids], axis=0)
    return out.reshape(B, S, N)


# revision 13
# speedup vs baseline: 1.0299x; 1.0299x over previous
"""FP8 fake-quant matmul on 8 TRN2 NeuronCores.

Computes reference semantics:
    w_dq = fq(weight, s_w);  x_dq = fq(x.reshape(-1,K), s_x)
    out  = (x_dq @ w_dq).reshape(B, S, N)
where fq(t, s) = clip(t*s, +-448) round-tripped through float8_e4m3fn (OCP),
s = 448 / amax(|t|).

Device strategy (data-parallel over rows M = B*S, 8 shards, one per core):
  Launch A: per-core partial amax of its x shard and weight shard (DVE
            abs-max reduce); host max-combines the per-core partials into the
            exact global fp32 amaxes and computes the scales (the cross-shard
            all-reduce of the sharding hint, done on host since it is 16
            floats).
  Launch B: per-core quantize + DoubleRow fp8 matmul + dequant.
    - TRN fp8e4 max-normal is 240, not OCP e4m3fn's 448, so quantization runs
      at HALF the reference scale: |x|*s/2 <= 224 needs no clip and rounds
      identically to OCP at full scale (only the subnormal tail differs,
      negligibly); dequant multiplies by 4/(s_x*s_w).
    - x arrives pre-transposed per shard (host layout prep) so both operands
      DMA with k on the partition axis; no on-chip transposes.
    - Weights stream through SBUF in 512-column groups, quantized on ACT;
      quantized x^T (fp8) is resident; matmuls run as 8 column sweeps of
      DoubleRow fp8 (256-deep contraction per instruction), accumulating in
      PSUM, with the first two sweeps interleaved in m-halves so the PE
      never waits on the x stream.
"""

import sys

for _p in ("/opt/trn_rl_repo", "/root/.axon_site"):
    if _p not in sys.path:
        sys.path.insert(0, _p)

import numpy as np

import concourse.bass as bass  # noqa: F401  (registers engine classes)
import concourse.tile as tile
from concourse import bacc, mybir
from concourse.bass_utils import run_bass_kernel_spmd

# Problem shapes (hardcoded per spec)
B, S, K, N = 8, 2048, 4096, 4096
NCORES = 8
MS = (B * S) // NCORES  # 2048 rows of x per core
WS = K // NCORES  # 512 rows of weight per core (amax sharding)
P = 128
FP32 = mybir.dt.float32
FP8 = mybir.dt.float8e4
FP8_MAX = np.float32(448.0)

_CACHE = {}


def _build_amax():
    nc = bacc.Bacc(None, target_bir_lowering=False, debug=False)
    xs = nc.declare_dram_parameter("xs", [MS, K], FP32, isOutput=False)
    ws = nc.declare_dram_parameter("ws", [WS, K], FP32, isOutput=False)
    pm = nc.declare_dram_parameter("pm", [P, 2], FP32, isOutput=True)
    nxt = MS // P  # 16
    nwt = WS // P  # 4
    with tile.TileContext(nc) as tc:
        with (
            tc.tile_pool(name="io", bufs=4) as io,
            tc.tile_pool(name="st", bufs=1) as stp,
        ):
            st = stp.tile([P, nxt + nwt], FP32)
            fin = stp.tile([P, 2], FP32)
            xt = xs[:].rearrange("(t p) k -> t p k", p=P)
            wt = ws[:].rearrange("(t p) k -> t p k", p=P)
            for i in range(nxt):
                t = io.tile([P, K], FP32, tag="io")
                nc.sync.dma_start(out=t[:], in_=xt[i])
                nc.vector.reduce_max(
                    st[:, i : i + 1], t[:], axis=mybir.AxisListType.X,
                    apply_absolute_value=True,
                )
            for i in range(nwt):
                t = io.tile([P, K], FP32, tag="io")
                nc.sync.dma_start(out=t[:], in_=wt[i])
                nc.vector.reduce_max(
                    st[:, nxt + i : nxt + i + 1], t[:], axis=mybir.AxisListType.X,
                    apply_absolute_value=True,
                )
            nc.vector.reduce_max(
                fin[:, 0:1], st[:, 0:nxt], axis=mybir.AxisListType.X
            )
            nc.vector.reduce_max(
                fin[:, 1:2], st[:, nxt : nxt + nwt], axis=mybir.AxisListType.X
            )
            nc.sync.dma_start(out=pm[:], in_=fin[:])
    nc.compile()
    return nc


def _build_main():
    """Launch B: quantize + DoubleRow fp8 matmul.

    Takes x pre-transposed on the host (xt = x_shard.T, [K, MS] row-major) so
    both operands DMA with k on the partition axis; no on-chip transposes.
    """
    nc = bacc.Bacc(None, target_bir_lowering=False, debug=False)
    xT = nc.declare_dram_parameter("xT", [K, MS], FP32, isOutput=False)
    w = nc.declare_dram_parameter("w", [K, N], FP32, isOutput=False)
    sc = nc.declare_dram_parameter("sc", [1, 8], FP32, isOutput=False)
    out = nc.declare_dram_parameter("out", [MS, N], FP32, isOutput=True)
    MT, KT = MS // P, K // P  # 16, 32
    CT = KT // 2  # 16 DoubleRow chunks of 256 contraction rows
    NB = 512  # psum bank width (fp32)
    NT = N // NB  # 8 column sweeps
    MB = 512  # x m-strip width
    MST = MS // MB  # 4 strips
    DR = mybir.MatmulPerfMode.DoubleRow
    with tile.TileContext(nc) as tc:
        with (
            tc.tile_pool(name="const", bufs=1) as cst,
            tc.tile_pool(name="wf", bufs=8) as wfp,
            tc.tile_pool(name="wq", bufs=3 * CT) as wqp,
            tc.tile_pool(name="xf", bufs=8) as xfp,
            tc.tile_pool(name="xq", bufs=CT) as xqp,
            tc.tile_pool(name="ob", bufs=4) as obp,
            tc.tile_pool(name="mps", bufs=6, space="PSUM") as mpsp,
        ):
            scs = cst.tile([P, 8], FP32)
            nc.sync.dma_start(out=scs[:], in_=sc[:].to_broadcast([P, 8]))
            sxs = scs[:, 0:1]  # s_x / 2
            sws = scs[:, 1:2]  # s_w / 2
            dqs = scs[:, 2:3]  # 4 / (s_x * s_w) with reference rounding

            # DoubleRow pairing: chunk c, plane i, partition p <-> k row
            # c*256 + i*128 + p, for both operands.
            w4 = w[:].rearrange("(c i p) n -> c p i n", i=2, p=P)  # [16,128,2,N]
            x4 = xT[:].rearrange("(c i p) m -> c p i m", i=2, p=P)  # [16,128,2,MS]
            ot = out[:].rearrange("(t p) n -> t p n", p=P)

            # Quantized x^T: resident, one tile per 256-row chunk.
            xqs = [
                xqp.tile([P, 2, MS], FP8, tag="xq", name=f"xq_{c}")
                for c in range(CT)
            ]

            def emit_xstrip(ms):
                # loads + quantizes x^T columns [ms*MB, (ms+1)*MB) for all chunks
                for c in range(CT):
                    xf = xfp.tile([P, 2, MB], FP32, tag="xf", name=f"xf_{ms}_{c}")
                    nc.sync.dma_start(
                        out=xf[:], in_=x4[c][:, :, ms * MB : (ms + 1) * MB]
                    )
                    if c % 2:
                        nc.scalar.mul(
                            xqs[c][:, :, ms * MB : (ms + 1) * MB], xf[:], sxs
                        )
                    else:
                        nc.vector.tensor_scalar_mul(
                            xqs[c][:, :, ms * MB : (ms + 1) * MB], xf[:], sxs
                        )

            wgroups = {}

            def emit_wgroup(j):
                tiles = []
                for c in range(CT):
                    wf = wfp.tile([P, 2, NB], FP32, tag="wf", name=f"wf_{j}_{c}")
                    nc.sync.dma_start(
                        out=wf[:], in_=w4[c][:, :, j * NB : (j + 1) * NB]
                    )
                    wq = wqp.tile([P, 2, NB], FP8, tag="wq", name=f"wq_{j}_{c}")
                    nc.scalar.mul(wq[:, :, :], wf[:, :, :], sws)
                    tiles.append(wq)
                wgroups[j] = tiles

            emit_xstrip(0)
            emit_wgroup(0)
            emit_xstrip(1)
            emit_wgroup(1)
            emit_xstrip(2)
            emit_xstrip(3)

            def mm_block(j, ms):
                for m in ms:
                    psum = mpsp.tile([P, NB], FP32, tag="mps", name=f"mps_{j}_{m}")
                    for c in range(CT):
                        nc.tensor.matmul(
                            psum[:],
                            xqs[c][:, :, m * P : (m + 1) * P],
                            wgroups[j][c][:, :, :],
                            start=(c == 0),
                            stop=(c == CT - 1),
                            perf_mode=DR,
                        )
                    ob = obp.tile([P, NB], FP32, tag="ob", name=f"ob_{j}_{m}")
                    nc.vector.tensor_scalar_mul(ob[:], psum[:], dqs)
                    nc.sync.dma_start(out=ot[m, :, j * NB : (j + 1) * NB], in_=ob[:])

            # First two column sweeps run in m-halves so the PE always has
            # matmuls whose x strips have already arrived; x strips 2-3 and
            # wgroup prefetches stream underneath.
            lo, hi = range(0, MT // 2), range(MT // 2, MT)
            mm_block(0, lo)
            mm_block(1, lo)
            emit_wgroup(2)
            mm_block(0, hi)
            del wgroups[0]
            emit_wgroup(3)
            mm_block(1, hi)
            del wgroups[1]
            for j in range(2, NT):
                for half, ms in ((0, lo), (1, hi)):
                    mm_block(j, ms)
                    if half == 0 and j + 2 < NT:
                        emit_wgroup(j + 2)
                del wgroups[j]
    nc.compile()
    return nc


def _get(name, builder):
    if name not in _CACHE:
        _CACHE[name] = builder()
    return _CACHE[name]


def kernel(x: np.ndarray, weight: np.ndarray) -> np.ndarray:
    x = np.ascontiguousarray(np.asarray(x, dtype=np.float32))
    weight = np.ascontiguousarray(np.asarray(weight, dtype=np.float32))
    assert x.shape == (B, S, K) and weight.shape == (K, N)
    x2d = x.reshape(B * S, K)

    core_ids = list(range(NCORES))
    x_shards = [x2d[c * MS : (c + 1) * MS] for c in core_ids]
    w_shards = [weight[c * WS : (c + 1) * WS] for c in core_ids]

    # ---- Launch A: partial amax ----
    nc_a = _get("amax", _build_amax)
    res_a = run_bass_kernel_spmd(
        nc_a,
        [{"xs": x_shards[c], "ws": w_shards[c]} for c in core_ids],
        core_ids,
    )
    pms = np.stack([res_a.results[c]["pm"] for c in core_ids])  # [8, 128, 2]
    amax_x = np.float32(pms[:, :, 0].max())
    amax_w = np.float32(pms[:, :, 1].max())

    # Exact reference scale arithmetic (fp32 throughout)
    s_x = FP8_MAX / np.maximum(amax_x, np.float32(1e-12))
    s_w = FP8_MAX / np.maximum(amax_w, np.float32(1e-12))
    r_x = np.float32(1.0) / s_x
    r_w = np.float32(1.0) / s_w
    dq = np.float32(4.0) * r_x * r_w
    scales = np.zeros((1, 8), np.float32)
    scales[0, 0] = s_x * np.float32(0.5)
    scales[0, 1] = s_w * np.float32(0.5)
    scales[0, 2] = dq

    # ---- Launch B: quantize + matmul (x pre-transposed per shard on host) ----
    xT_shards = [np.ascontiguousarray(s.T) for s in x_shards]
    nc_b = _get("main", _build_main)
    res_b = run_bass_kernel_spmd(
        nc_b,
        [{"xT": xT_shards[c], "w": weight, "sc": scales} for c in core_ids],
        core_ids,
    )
    out = np.concatenate([res_b.results[c]["out"] for c in core_ids], axis=0)
    return out.reshape(B, S, N)


# revision 15
# speedup vs baseline: 1.0621x; 1.0313x over previous
"""FP8 fake-quant matmul on 8 TRN2 NeuronCores.

Computes reference semantics:
    w_dq = fq(weight, s_w);  x_dq = fq(x.reshape(-1,K), s_x)
    out  = (x_dq @ w_dq).reshape(B, S, N)
where fq(t, s) = clip(t*s, +-448) round-tripped through float8_e4m3fn (OCP),
s = 448 / amax(|t|).

Device strategy (data-parallel over rows M = B*S, 8 shards, one per core):
  Launch A: per-core partial amax of its x shard and weight shard (DVE
            abs-max reduce); host max-combines the per-core partials into the
            exact global fp32 amaxes and computes the scales (the cross-shard
            all-reduce of the sharding hint, done on host since it is 16
            floats).
  Launch B: per-core quantize + DoubleRow fp8 matmul + dequant.
    - TRN fp8e4 max-normal is 240, not OCP e4m3fn's 448, so quantization runs
      at HALF the reference scale: |x|*s/2 <= 224 needs no clip and rounds
      identically to OCP at full scale (only the subnormal tail differs,
      negligibly); dequant multiplies by 4/(s_x*s_w).
    - x arrives pre-transposed per shard (host layout prep) so both operands
      DMA with k on the partition axis; no on-chip transposes.
    - Weights stream through SBUF in 512-column groups, quantized on ACT;
      quantized x^T (fp8) is resident; matmuls run as 8 column sweeps of
      DoubleRow fp8 (256-deep contraction per instruction), accumulating in
      PSUM, with the first two sweeps interleaved in m-halves so the PE
      never waits on the x stream.
"""

import sys

for _p in ("/opt/trn_rl_repo", "/root/.axon_site"):
    if _p not in sys.path:
        sys.path.insert(0, _p)

import numpy as np

import concourse.bass as bass  # noqa: F401  (registers engine classes)
import concourse.tile as tile
from concourse import bacc, mybir
from concourse.bass_utils import run_bass_kernel_spmd

# Problem shapes (hardcoded per spec)
B, S, K, N = 8, 2048, 4096, 4096
NCORES = 8
MS = (B * S) // NCORES  # 2048 rows of x per core
WS = K // NCORES  # 512 rows of weight per core (amax sharding)
P = 128
FP32 = mybir.dt.float32
FP8 = mybir.dt.float8e4
FP8_MAX = np.float32(448.0)

_CACHE = {}


def _build_amax():
    nc = bacc.Bacc(None, target_bir_lowering=False, debug=False)
    xs = nc.declare_dram_parameter("xs", [MS, K], FP32, isOutput=False)
    ws = nc.declare_dram_parameter("ws", [WS, K], FP32, isOutput=False)
    pm = nc.declare_dram_parameter("pm", [P, 2], FP32, isOutput=True)
    nxt = MS // P  # 16
    nwt = WS // P  # 4
    with tile.TileContext(nc) as tc:
        with (
            tc.tile_pool(name="io", bufs=4) as io,
            tc.tile_pool(name="st", bufs=1) as stp,
        ):
            st = stp.tile([P, nxt + nwt], FP32)
            fin = stp.tile([P, 2], FP32)
            xt = xs[:].rearrange("(t p) k -> t p k", p=P)
            wt = ws[:].rearrange("(t p) k -> t p k", p=P)
            for i in range(nxt):
                t = io.tile([P, K], FP32, tag="io")
                nc.sync.dma_start(out=t[:], in_=xt[i])
                nc.vector.reduce_max(
                    st[:, i : i + 1], t[:], axis=mybir.AxisListType.X,
                    apply_absolute_value=True,
                )
            for i in range(nwt):
                t = io.tile([P, K], FP32, tag="io")
                nc.sync.dma_start(out=t[:], in_=wt[i])
                nc.vector.reduce_max(
                    st[:, nxt + i : nxt + i + 1], t[:], axis=mybir.AxisListType.X,
                    apply_absolute_value=True,
                )
            nc.vector.reduce_max(
                fin[:, 0:1], st[:, 0:nxt], axis=mybir.AxisListType.X
            )
            nc.vector.reduce_max(
                fin[:, 1:2], st[:, nxt : nxt + nwt], axis=mybir.AxisListType.X
            )
            nc.sync.dma_start(out=pm[:], in_=fin[:])
    nc.compile()
    return nc


def _build_main():
    """Launch B: quantize + DoubleRow fp8 matmul.

    Takes x pre-transposed on the host (xt = x_shard.T, [K, MS] row-major) so
    both operands DMA with k on the partition axis; no on-chip transposes.
    """
    nc = bacc.Bacc(None, target_bir_lowering=False, debug=False)
    xT = nc.declare_dram_parameter("xT", [K, MS], FP32, isOutput=False)
    w = nc.declare_dram_parameter("w", [K, N], FP32, isOutput=False)
    sc = nc.declare_dram_parameter("sc", [1, 8], FP32, isOutput=False)
    out = nc.declare_dram_parameter("out", [MS, N], FP32, isOutput=True)
    MT, KT = MS // P, K // P  # 16, 32
    CT = KT // 2  # 16 DoubleRow chunks of 256 contraction rows
    NB = 512  # psum bank width (fp32)
    NT = N // NB  # 8 column sweeps
    MB = 512  # x m-strip width
    MST = MS // MB  # 4 strips
    DR = mybir.MatmulPerfMode.DoubleRow
    with tile.TileContext(nc) as tc:
        with (
            tc.tile_pool(name="const", bufs=1) as cst,
            tc.tile_pool(name="wf", bufs=8) as wfp,
            tc.tile_pool(name="wq", bufs=3 * CT) as wqp,
            tc.tile_pool(name="xf", bufs=8) as xfp,
            tc.tile_pool(name="xq", bufs=CT) as xqp,
            tc.tile_pool(name="ob", bufs=4) as obp,
            tc.tile_pool(name="mps", bufs=6, space="PSUM") as mpsp,
        ):
            scs = cst.tile([P, 8], FP32)
            nc.sync.dma_start(out=scs[:], in_=sc[:].to_broadcast([P, 8]))
            sxs = scs[:, 0:1]  # s_x / 2
            sws = scs[:, 1:2]  # s_w / 2
            dqs = scs[:, 2:3]  # 4 / (s_x * s_w) with reference rounding

            # DoubleRow pairing: chunk c, plane i, partition p <-> k row
            # c*256 + i*128 + p, for both operands.
            w4 = w[:].rearrange("(c i p) n -> c p i n", i=2, p=P)  # [16,128,2,N]
            x4 = xT[:].rearrange("(c i p) m -> c p i m", i=2, p=P)  # [16,128,2,MS]
            ot = out[:].rearrange("(t p) n -> t p n", p=P)

            # Quantized x^T: resident, one tile per 256-row chunk.
            xqs = [
                xqp.tile([P, 2, MS], FP8, tag="xq", name=f"xq_{c}")
                for c in range(CT)
            ]

            def emit_xchunk(ms, c):
                # loads + quantizes x^T chunk c, columns [ms*MB, (ms+1)*MB)
                xf = xfp.tile([P, 2, MB], FP32, tag="xf", name=f"xf_{ms}_{c}")
                nc.sync.dma_start(
                    out=xf[:], in_=x4[c][:, :, ms * MB : (ms + 1) * MB]
                )
                if c % 2:
                    nc.scalar.mul(
                        xqs[c][:, :, ms * MB : (ms + 1) * MB], xf[:], sxs
                    )
                else:
                    nc.vector.tensor_scalar_mul(
                        xqs[c][:, :, ms * MB : (ms + 1) * MB], xf[:], sxs
                    )

            def emit_xstrip(ms):
                for c in range(CT):
                    emit_xchunk(ms, c)

            wgroups = {}
            wtiles = {}

            def emit_wchunk(j, c):
                wf = wfp.tile([P, 2, NB], FP32, tag="wf", name=f"wf_{j}_{c}")
                nc.sync.dma_start(
                    out=wf[:], in_=w4[c][:, :, j * NB : (j + 1) * NB]
                )
                wq = wqp.tile([P, 2, NB], FP8, tag="wq", name=f"wq_{j}_{c}")
                nc.scalar.mul(wq[:, :, :], wf[:, :, :], sws)
                wtiles.setdefault(j, []).append(wq)

            def emit_wgroup(j):
                for c in range(CT):
                    emit_wchunk(j, c)
                wgroups[j] = wtiles[j]

            # Emission order == DMA queue order == consumption order:
            # strip0+wg0 interleaved (gates the very first matmuls), then
            # strip1, wg1, strips 2-3; early matmuls run in strip-sized
            # quarter blocks so the PE always has work whose data arrived.
            for c in range(CT):
                emit_xchunk(0, c)
                emit_wchunk(0, c)
            wgroups[0] = wtiles[0]
            emit_xstrip(1)
            emit_wgroup(1)
            emit_xstrip(2)
            emit_xstrip(3)

            def mm_block(j, ms):
                for m in ms:
                    psum = mpsp.tile([P, NB], FP32, tag="mps", name=f"mps_{j}_{m}")
                    for c in range(CT):
                        nc.tensor.matmul(
                            psum[:],
                            xqs[c][:, :, m * P : (m + 1) * P],
                            wgroups[j][c][:, :, :],
                            start=(c == 0),
                            stop=(c == CT - 1),
                            perf_mode=DR,
                        )
                    ob = obp.tile([P, NB], FP32, tag="ob", name=f"ob_{j}_{m}")
                    nc.vector.tensor_scalar_mul(ob[:], psum[:], dqs)
                    nc.sync.dma_start(out=ot[m, :, j * NB : (j + 1) * NB], in_=ob[:])

            def quarter(q):
                return range(4 * q, 4 * q + 4)

            mm_block(0, quarter(0))
            mm_block(0, quarter(1))
            mm_block(1, quarter(0))
            mm_block(1, quarter(1))
            emit_wgroup(2)
            mm_block(0, quarter(2))
            mm_block(1, quarter(2))
            mm_block(0, quarter(3))
            mm_block(1, quarter(3))
            emit_wgroup(3)
            for j in range(2, NT):
                mm_block(j, range(0, MT // 2))
                if j + 2 < NT:
                    emit_wgroup(j + 2)
                mm_block(j, range(MT // 2, MT))
                del wgroups[j]
    nc.compile()
    return nc


def _get(name, builder):
    if name not in _CACHE:
        _CACHE[name] = builder()
    return _CACHE[name]


def kernel(x: np.ndarray, weight: np.ndarray) -> np.ndarray:
    x = np.ascontiguousarray(np.asarray(x, dtype=np.float32))
    weight = np.ascontiguousarray(np.asarray(weight, dtype=np.float32))
    assert x.shape == (B, S, K) and weight.shape == (K, N)
    x2d = x.reshape(B * S, K)

    core_ids = list(range(NCORES))
    x_shards = [x2d[c * MS : (c + 1) * MS] for c in core_ids]
    w_shards = [weight[c * WS : (c + 1) * WS] for c in core_ids]

    # ---- Launch A: partial amax ----
    nc_a = _get("amax", _build_amax)
    res_a = run_bass_kernel_spmd(
        nc_a,
        [{"xs": x_shards[c], "ws": w_shards[c]} for c in core_ids],
        core_ids,
    )
    pms = np.stack([res_a.results[c]["pm"] for c in core_ids])  # [8, 128, 2]
    amax_x = np.float32(pms[:, :, 0].max())
    amax_w = np.float32(pms[:, :, 1].max())

    # Exact reference scale arithmetic (fp32 throughout)
    s_x = FP8_MAX / np.maximum(amax_x, np.float32(1e-12))
    s_w = FP8_MAX / np.maximum(amax_w, np.float32(1e-12))
    r_x = np.float32(1.0) / s_x
    r_w = np.float32(1.0) / s_w
    dq = np.float32(4.0) * r_x * r_w
    scales = np.zeros((1, 8), np.float32)
    scales[0, 0] = s_x * np.float32(0.5)
    scales[0, 1] = s_w * np.float32(0.5)
    scales[0, 2] = dq

    # ---- Launch B: quantize + matmul (x pre-transposed per shard on host) ----
    xT_shards = [np.ascontiguousarray(s.T) for s in x_shards]
    nc_b = _get("main", _build_main)
    res_b = run_bass_kernel_spmd(
        nc_b,
        [{"xT": xT_shards[c], "w": weight, "sc": scales} for c in core_ids],
        core_ids,
    )
    out = np.concatenate([res_b.results[c]["out"] for c in core_ids], axis=0)
    return out.reshape(B, S, N)
